# revision 1
# baseline (speedup 1.0000x reference)
"""
Self-contained Bass/Trainium2 kernel for the 2-layer 2-head GAT
(nn_GATNet): kernel(**inputs) takes the FULL unsharded inputs
(x [50000,128] f32, edge_index [2,800000] int64, W0, attn0, W1, attn1)
and returns the FULL [50000, 1] f32 output, computed on 8 TRN2
NeuronCores via bass_utils.run_bass_kernel_spmd.

See the strategy notes below (destination-node sharding, padded-CSR
dma_gather batches over AllGathered node tables, on-chip segment
softmax without max-subtraction).
"""
import sys
if "/opt/trn_rl_repo" not in sys.path:
    sys.path.insert(0, "/opt/trn_rl_repo")

"""
GAT (2-layer, 2-head) Bass/Tile kernel for TRN2, 8-core SPMD.

Strategy
--------
Destination-node sharding: core r owns destination nodes [r*S, (r+1)*S).
All edges pointing into that slice are processed by that core, so the
segment softmax and the weighted scatter are purely core-local; the only
collectives are AllGathers of the (node-major) gather tables.

Per layer, node records live in a DRAM table in *permuted position* order
(within each core's slice, nodes are sorted by (deg_lo, deg_hi) so that
padded-CSR batches are uniform):
  L0: Htab [N, 192] f32  (row = [h0(128), a_c0(2), a_r0(2), junk], 768B)
  L1: T1tab [N, 64] f32  (row = [h1(2), a_c1(2), a_r1(2), 0...], 256B)

Edge phase (per batch of 128 destination nodes, K slots per node):
  G = dma_gather(table, idx)                  # [128, K, rec] (4 SWDGE queues)
  t = a_c(G) + a_r(dest) + mneg               # mneg = 0 real / -1e30 pad
  w = exp(max(t, 0.2*t))                      # softmax numerator
                                              # (no max-subtract: shift-inv)
  s = sum_k w ; acc = sum_k w * feat(G)
  out = (acc_h0/max(s_h0,eps) + acc_h1/max(s_h1,eps)) / 2   # head mean

The softmax max-subtraction is dropped (mathematically identical; |t|<~10
for this data so no overflow).  Padding slots gather row 0 and get
t = -1e30 from the additive mask => w == 0 exactly.

int16 gather indices only reach 32767, so each batch gathers twice:
a lo half (table rows 0..N/2-1) and a hi half (rows N/2..N-1).
"""

import math
from contextlib import ExitStack
from dataclasses import dataclass, field

import numpy as np

import concourse.bass as bass
import concourse.bacc as bacc
import concourse.mybir as mybir
import concourse.tile as tile
from concourse import masks

F32 = mybir.dt.float32
I16 = mybir.dt.int16

IN_CH = 128
HID = 64
HEADS = 2
OUT_CH = 1
REC0 = 192                  # 768B gather record: [h0(128), a_c(2), a_r(2), x]
REC1 = 64                   # 256B gather record for layer 1
NEG = -1.0e30


# ----------------------------------------------------------------------------
# Host-side preprocessing
# ----------------------------------------------------------------------------

@dataclass
class Schedule:
    N: int
    NC: int
    S: int                      # nodes per core
    B: int                      # batches per core
    S_pad: int                  # B*128
    half: int                   # N // 2
    K_lo: list = field(default_factory=list)   # per-batch lo slot count
    K_hi: list = field(default_factory=list)
    off_lo: list = field(default_factory=list)  # free-dim offset into idx tile
    off_hi: list = field(default_factory=list)
    moff: list = field(default_factory=list)    # free-dim offset into mask tile
    W: int = 0                  # idx tile free dim
    MW: int = 0                 # mask tile free dim
    items: list = field(default_factory=list)   # K-capped pipeline items


def build_host_data(x, edge_index, W0, attn0, W1, attn1, NC=8):
    """Returns (schedule, per_core_inputs, unperm) where unperm[g] is the
    original node id at permuted global position g."""
    x = np.asarray(x, np.float32)
    edge_index = np.asarray(edge_index)
    W0 = np.asarray(W0, np.float32)
    attn0 = np.asarray(attn0, np.float32)
    W1 = np.asarray(W1, np.float32)
    attn1 = np.asarray(attn1, np.float32)

    N = x.shape[0]
    assert N % (2 * NC) == 0, (N, NC)
    S = N // NC
    B = (S + 127) // 128
    S_pad = B * 128
    half = N // 2

    row = edge_index[0].astype(np.int64)
    col = edge_index[1].astype(np.int64)

    sched = Schedule(N=N, NC=NC, S=S, B=B, S_pad=S_pad, half=half)

    # ---- per-core permutation (sort by (deg_lo, deg_hi)) ----
    perms = []
    pos = np.empty(N, np.int64)
    core_edges = []
    for r in range(NC):
        lo_n, hi_n = r * S, (r + 1) * S
        m = (row >= lo_n) & (row < hi_n)
        er, ec = row[m] - lo_n, col[m]
        elo = ec < half
        deg_lo = np.bincount(er[elo], minlength=S)
        deg_hi = np.bincount(er[~elo], minlength=S)
        key = deg_lo.astype(np.int64) * 100000 + deg_hi
        perm = np.argsort(key, kind="stable")
        rank_of = np.empty(S, np.int64)
        rank_of[perm] = np.arange(S)
        perms.append(perm)
        pos[lo_n:hi_n] = r * S + rank_of
        core_edges.append((rank_of[er], ec, elo))

    unperm = np.empty(N, np.int64)
    unperm[pos] = np.arange(N)

    # ---- per-core padded-CSR slot layout (shared K per batch across cores) --
    all_lists = []
    deg_lo_mat = np.zeros((NC, S_pad), np.int64)
    deg_hi_mat = np.zeros((NC, S_pad), np.int64)
    for r in range(NC):
        er, ec, elo = core_edges[r]
        src_pos = pos[ec]
        lists_lo = [[] for _ in range(S_pad)]
        lists_hi = [[] for _ in range(S_pad)]
        for q, p, lo in zip(er, src_pos, elo):
            (lists_lo if lo else lists_hi)[q].append(p)
        for q in range(S):
            deg_lo_mat[r, q] = len(lists_lo[q])
            deg_hi_mat[r, q] = len(lists_hi[q])
        all_lists.append((lists_lo, lists_hi))

    off = 0
    moff = 0
    for b in range(B):
        sl = slice(b * 128, (b + 1) * 128)
        klo = int(deg_lo_mat[:, sl].max(axis=1).max())
        khi = int(deg_hi_mat[:, sl].max(axis=1).max())
        sched.K_lo.append(klo)
        sched.K_hi.append(khi)
        sched.off_lo.append(off)
        off += 8 * klo
        sched.off_hi.append(off)
        off += 8 * khi
        sched.moff.append(moff)
        moff += klo + khi
    sched.W = max(off, 16)
    sched.MW = max(moff, 1)

    # split heavy batches into pipeline items with K <= KCAP
    # item: (b, klo_s, klo_n, khi_s, khi_n, first, last) where the slices are
    # k-ranges within the batch's lo/hi slot blocks.
    KCAP = 26
    items = []
    for b in range(B):
        klo, khi = sched.K_lo[b], sched.K_hi[b]
        cuts = []
        done_lo = done_hi = 0
        while done_lo < klo or done_hi < khi:
            take_lo = min(KCAP, klo - done_lo)
            take_hi = min(KCAP - take_lo, khi - done_hi)
            cuts.append((done_lo, take_lo, done_hi, take_hi))
            done_lo += take_lo
            done_hi += take_hi
        for i, (ls, ln, hs, hn) in enumerate(cuts):
            items.append(dict(b=b, klo_s=ls, klo_n=ln, khi_s=hs, khi_n=hn,
                              first=(i == 0), last=(i == len(cuts) - 1)))
    sched.items = items

    # per-item mask offsets (item-ordered contiguous [lo_n | hi_n] blocks)
    imoff = 0
    for it in sched.items:
        it["moff"] = imoff
        imoff += it["klo_n"] + it["khi_n"]
    assert imoff == sched.MW or sched.MW == 1

    # ---- build per-core idx / mneg tiles + permuted x slices ----
    per_core = []
    for r in range(NC):
        lists_lo, lists_hi = all_lists[r]
        idx = np.zeros((16, sched.W), np.int16)
        mneg = np.full((128, sched.MW), NEG, np.float32)
        for b in range(B):
            for (kk, offs, lists, is_hi) in (
                (sched.K_lo[b], sched.off_lo[b], lists_lo, False),
                (sched.K_hi[b], sched.off_hi[b], lists_hi, True),
            ):
                if kk == 0:
                    continue
                blk = np.zeros(kk * 128, np.int64)
                for p in range(128):
                    lst = lists[b * 128 + p]
                    for k, src in enumerate(lst):
                        blk[k * 128 + p] = (src - half) if is_hi else src
                assert blk.max() <= 32767
                idx[:, offs:offs + 8 * kk] = blk.reshape(8 * kk, 16).T
        # item-ordered additive mask
        for it in sched.items:
            b = it["b"]
            col = it["moff"]
            for (ks, kn, lists) in ((it["klo_s"], it["klo_n"], lists_lo),
                                    (it["khi_s"], it["khi_n"], lists_hi)):
                for p in range(128):
                    d = len(lists[b * 128 + p])
                    nreal = max(0, min(d - ks, kn))
                    mneg[p, col:col + nreal] = 0.0
                col += kn
        idx_tile = np.broadcast_to(
            idx[None, :, :], (8, 16, sched.W)).reshape(128, sched.W).copy()

        xp = np.zeros((S_pad, IN_CH), np.float32)
        xp[:S] = x[r * S + perms[r]]
        per_core.append({"xp": xp, "idx": np.ascontiguousarray(idx_tile),
                         "mneg": mneg})

    # ---- weights / constants (shared across cores) ----
    # wcat0 cols: [0:128]=W0(h0), [128:130]=a_c0 fold, [130:132]=a_r0 fold
    wcat0 = np.zeros((IN_CH, REC0), np.float32)
    wcat0[:, :128] = W0
    for h in range(HEADS):
        blk = W0[:, h * HID:(h + 1) * HID].astype(np.float64)
        wcat0[:, 128 + h] = (blk @ attn0[h, HID:].astype(np.float64)).astype(np.float32)
        wcat0[:, 130 + h] = (blk @ attn0[h, :HID].astype(np.float64)).astype(np.float32)

    # wcat1 cols: [0:2]=h1, [2:4]=a_c1 fold, [4:6]=a_r1 fold, rest 0
    wcat1 = np.zeros((HID, REC1), np.float32)
    wcat1[:, 0:2] = W1
    for h in range(HEADS):
        wcat1[:, 2 + h] = W1[:, h] * attn1[h, 1]
        wcat1[:, 4 + h] = W1[:, h] * attn1[h, 0]

    for d in per_core:
        d.update({"wcat0": wcat0, "wcat1": wcat1})
    return sched, per_core, unperm


# ----------------------------------------------------------------------------
# Numpy emulation of the device algorithm (for fast validation)
# ----------------------------------------------------------------------------

def emulate(sched, per_core, unperm):
    N, NC, B, half = sched.N, sched.NC, sched.B, sched.half
    S, S_pad = sched.S, sched.S_pad

    def lrelu_exp(t):
        return np.exp(np.maximum(t, 0.2 * t), dtype=np.float32)

    Htab = np.zeros((N, REC0), np.float32)
    Atab = np.zeros((NC, S_pad, HEADS), np.float32)
    for r in range(NC):
        d = per_core[r]
        hm = d["xp"] @ d["wcat0"]          # [S_pad, 192]
        Htab[r * S:(r + 1) * S] = hm[:S]
        Atab[r] = hm[:, 130:132]

    def edge_phase_item(r, it, tab, rec):
        d = per_core[r]
        idx = d["idx"][:16]
        K = it["klo_n"] + it["khi_n"]
        b = it["b"]
        G = np.zeros((128, K, rec), np.float32)
        for (ks, kn, offs, base, k0) in (
            (it["klo_s"], it["klo_n"], sched.off_lo[b], 0, 0),
            (it["khi_s"], it["khi_n"], sched.off_hi[b], half, it["klo_n"]),
        ):
            for j in range(kn * 128):
                jj = (ks + j // 128) * 128 + (j % 128)
                v = int(idx[jj % 16, offs + jj // 16])
                G[j % 128, k0 + j // 128] = tab[base + v]
        m = d["mneg"][:, it["moff"]:it["moff"] + K]
        return G, m

    x1 = np.zeros((NC, S_pad, HID), np.float32)
    for r in range(NC):
        sa = {}
        for it in sched.items:
            b = it["b"]
            K = it["klo_n"] + it["khi_n"]
            if K == 0:
                continue
            G, m = edge_phase_item(r, it, Htab, REC0)
            t = (G[:, :, 128:130] + Atab[r, b * 128:(b + 1) * 128, None, :]
                 + m[:, :, None])
            w = lrelu_exp(t)
            s_i = w.sum(1)
            acc_i = (G[:, :, :128].reshape(128, K, HEADS, HID)
                     * w[..., None]).sum(1)
            if it["first"]:
                sa[b] = (s_i, acc_i)
            else:
                sa[b] = (sa[b][0] + s_i, sa[b][1] + acc_i)
            if it["last"]:
                s, acc = sa.pop(b)
                s = np.maximum(s, 1e-30)
                x1[r, b * 128:(b + 1) * 128] = np.maximum(
                    0.5 * (acc[:, 0] / s[:, :1] + acc[:, 1] / s[:, 1:]), 0.0)

    T1 = np.zeros((N, REC1), np.float32)
    A1 = np.zeros((NC, S_pad, HEADS), np.float32)
    for r in range(NC):
        t1 = x1[r] @ per_core[r]["wcat1"]
        T1[r * S:(r + 1) * S] = t1[:S]
        A1[r] = t1[:, 4:6]

    out = np.zeros((NC, S_pad), np.float32)
    for r in range(NC):
        sa = {}
        for it in sched.items:
            b = it["b"]
            K = it["klo_n"] + it["khi_n"]
            if K == 0:
                continue
            G, m = edge_phase_item(r, it, T1, REC1)
            t = G[:, :, 2:4] + A1[r, b * 128:(b + 1) * 128, None, :] + m[:, :, None]
            w = lrelu_exp(t)
            s_i = w.sum(1)
            acc_i = (G[:, :, 0:2] * w).sum(1)
            if it["first"]:
                sa[b] = (s_i, acc_i)
            else:
                sa[b] = (sa[b][0] + s_i, sa[b][1] + acc_i)
            if it["last"]:
                s, acc = sa.pop(b)
                s = np.maximum(s, 1e-30)
                out[r, b * 128:(b + 1) * 128] = 0.5 * (acc[:, 0] / s[:, 0]
                                                       + acc[:, 1] / s[:, 1])

    full = out[:, :S].reshape(-1)
    res = np.empty((N, 1), np.float32)
    res[unperm] = full[:, None]
    return res


# ----------------------------------------------------------------------------
# Device kernel builder
# ----------------------------------------------------------------------------

def build_kernel(sched: Schedule, gbufs=4):
    N, NC, B, half, S, S_pad, W = (sched.N, sched.NC, sched.B, sched.half,
                                   sched.S, sched.S_pad, sched.W)
    nc = bacc.Bacc("TRN2", target_bir_lowering=False, debug=False,
                   num_devices=NC, num_swdge_queues=4)

    xp_d = nc.dram_tensor("xp", [S_pad, IN_CH], F32, kind="ExternalInput")
    idx_d = nc.dram_tensor("idx", [128, W], I16, kind="ExternalInput")
    mneg_d = nc.dram_tensor("mneg", [128, sched.MW], F32, kind="ExternalInput")
    wcat0_d = nc.dram_tensor("wcat0", [IN_CH, REC0], F32, kind="ExternalInput")
    wcat1_d = nc.dram_tensor("wcat1", [HID, REC1], F32, kind="ExternalInput")
    out_d = nc.dram_tensor("out", [128, B], F32, kind="ExternalOutput")

    rg = [list(range(NC))]
    qn = [0]

    def next_q():
        q = qn[0] % 4
        qn[0] += 1
        return q

    with tile.TileContext(nc) as tc, ExitStack() as ctx:
        aspace = "Shared" if NC > 4 else "Local"
        dram = ctx.enter_context(tc.tile_pool(name="dram", bufs=1, space="DRAM"))
        hslice = dram.tile([S, REC0], F32)
        htab = dram.tile([N, REC0], F32, addr_space=aspace)
        t1slice = dram.tile([S, REC1], F32)
        t1tab = dram.tile([N, REC1], F32, addr_space=aspace)

        const = ctx.enter_context(tc.tile_pool(name="const", bufs=1))
        wcat0 = const.tile([IN_CH, REC0], F32)
        wcat1 = const.tile([HID, REC1], F32)
        ident = const.tile([128, 128], F32)
        idx_sb = const.tile([128, W], I16)
        mneg_sb = const.tile([128, sched.MW], F32)
        a0_sb = const.tile([128, B, HEADS], F32)
        a1_sb = const.tile([128, B, HEADS], F32)
        out_sb = const.tile([128, B], F32)
        x1_all = const.tile([128, B, HID], F32)

        nc.sync.dma_start(wcat0[:, :], wcat0_d[:, :])
        nc.sync.dma_start(wcat1[:, :], wcat1_d[:, :])
        nc.sync.dma_start(idx_sb[:, :], idx_d[:, :])
        nc.sync.dma_start(mneg_sb[:, :], mneg_d[:, :])
        masks.make_identity(nc, ident[:, :])

        xin = ctx.enter_context(tc.tile_pool(name="xin", bufs=3))
        stage = ctx.enter_context(tc.tile_pool(name="stage", bufs=3))
        psum = ctx.enter_context(tc.tile_pool(name="psum", bufs=2, space="PSUM"))

        # ---------------- phase 1: H table (layer-0 node matmul) -------------
        for t in range(B):
            rows = min(128, S - t * 128)
            x_t = xin.tile([128, IN_CH], F32, tag="x")
            nc.sync.dma_start(x_t[:, :], xp_d[t * 128:(t + 1) * 128, :])
            ps_tr = psum.tile([128, 128], F32, tag="tp")
            nc.tensor.transpose(ps_tr[:, :], x_t[:, :], ident[:, :])
            xt = stage.tile([128, 128], F32, tag="xt")
            nc.vector.tensor_copy(xt[:, :], ps_tr[:, :])
            ps_mm = psum.tile([128, REC0], F32, tag="mm")
            nc.tensor.matmul(ps_mm[:, :], xt[:, :], wcat0[:, :],
                             start=True, stop=True)
            ht = stage.tile([128, REC0], F32, tag="ht")
            nc.vector.tensor_copy(ht[:, :], ps_mm[:, :])
            nc.vector.tensor_copy(a0_sb[:, t, :], ps_mm[:, 130:132])
            nc.sync.dma_start(hslice[t * 128:t * 128 + rows, :],
                              ht[0:rows, :])

        nc.gpsimd.collective_compute(
            "AllGather", mybir.AluOpType.bypass, replica_groups=rg,
            ins=[hslice[:, :]], outs=[htab[:, :]])

        # ---------------- edge-phase machinery -------------------------------
        gpool = ctx.enter_context(tc.tile_pool(name="gpool", bufs=gbufs))
        ppool = ctx.enter_context(tc.tile_pool(name="ppool", bufs=3))
        g1pool = ctx.enter_context(tc.tile_pool(name="g1pool", bufs=5))
        small = ctx.enter_context(tc.tile_pool(name="small", bufs=4))
        accp = ctx.enter_context(tc.tile_pool(name="accp", bufs=3))

        def gathers(g, table, rec, it):
            b = it["b"]
            klo, khi = it["klo_n"], it["khi_n"]
            if klo:
                o = sched.off_lo[b] + 8 * it["klo_s"]
                nc.gpsimd.dma_gather(
                    g[:, 0:klo, :], table[0:half, :],
                    idx_sb[:, o:o + 8 * klo],
                    num_idxs=128 * klo, num_idxs_reg=128 * klo, elem_size=rec,
                    single_packet=False, queue_num=next_q())
            if khi:
                o = sched.off_hi[b] + 8 * it["khi_s"]
                nc.gpsimd.dma_gather(
                    g[:, klo:klo + khi, :], table[half:N, :],
                    idx_sb[:, o:o + 8 * khi],
                    num_idxs=128 * khi, num_idxs_reg=128 * khi, elem_size=rec,
                    single_packet=False, queue_num=next_q())

        def softmax_w(g, it, K, ac_col, a_sb, rec):
            """t = a_c + a_r + mneg ; w = exp(max(t, .2t)); s_i per head."""
            b = it["b"]
            tt = small.tile([128, HEADS, K], F32, tag="tt", name="tt")
            for h in range(HEADS):
                nc.vector.scalar_tensor_tensor(
                    tt[:, h, :],
                    bass.AP(g.tensor, g.offset + ac_col + h, [g.ap[0], [rec, K]]),
                    a_sb[:, b, h:h + 1],
                    mneg_sb[:, it["moff"]:it["moff"] + K],
                    op0=mybir.AluOpType.add, op1=mybir.AluOpType.add)
            lr = small.tile([128, HEADS, K], F32, tag="lr", name="lr")
            nc.vector.scalar_tensor_tensor(
                lr[:, :, :], tt[:, :, :], 0.2, tt[:, :, :],
                op0=mybir.AluOpType.mult, op1=mybir.AluOpType.max)
            w = small.tile([128, HEADS, K], F32, tag="w", name="w")
            s_i = small.tile([128, HEADS], F32, tag="s", name="s_i")
            for h in range(HEADS):
                nc.scalar.activation(w[:, h, :], lr[:, h, :],
                                     mybir.ActivationFunctionType.Exp,
                                     accum_out=s_i[:, h:h + 1])
            return w, s_i

        # ---------------- phase 2: layer-0 edge phase ------------------------
        acc_of = {}
        for it in sched.items:
            b = it["b"]
            K = it["klo_n"] + it["khi_n"]
            if K == 0:
                nc.vector.memset(x1_all[:, b, :], 0.0)
                continue
            g = gpool.tile([128, K, REC0], F32, tag="g", name="g")
            gathers(g, htab, REC0, it)
            w, s_i = softmax_w(g, it, K, 128, a0_sb, REC0)

            pt2 = ppool.tile([128, K * 128], F32, tag="p", name="pt2")
            nc.vector.tensor_tensor(
                pt2[:, :].rearrange("p (k h c) -> p k h c", k=K, h=HEADS),
                bass.AP(g.tensor, g.offset,
                        [g.ap[0], [REC0, K], [HID, HEADS], [1, HID]]),
                w[:, :, :].rearrange("p h k -> p k h").unsqueeze(3)
                          .broadcast_to([128, K, HEADS, HID]),
                op=mybir.AluOpType.mult)
            if it["first"]:
                sacc = accp.tile([128, HEADS], F32, tag="sa", name="sacc")
                acc = accp.tile([128, HEADS, HID], F32, tag="aa", name="acc")
                acc_of[b] = (sacc, acc)
                nc.vector.tensor_copy(sacc[:, :], s_i[:, :])
                nc.vector.reduce_sum(
                    acc[:, :, :],
                    pt2[:, :].rearrange("p (k h c) -> p h c k", k=K, h=HEADS),
                    axis=mybir.AxisListType.X)
            else:
                sacc, acc = acc_of[b]
                nc.vector.tensor_add(sacc[:, :], sacc[:, :], s_i[:, :])
                acc_i = small.tile([128, HEADS, HID], F32, tag="ai", name="acc_i")
                nc.vector.reduce_sum(
                    acc_i[:, :, :],
                    pt2[:, :].rearrange("p (k h c) -> p h c k", k=K, h=HEADS),
                    axis=mybir.AxisListType.X)
                nc.vector.tensor_add(acc[:, :, :], acc[:, :, :], acc_i[:, :, :])
            if not it["last"]:
                continue
            sacc, acc = acc_of.pop(b)
            rs = small.tile([128, HEADS], F32, tag="rs", name="rs")
            nc.vector.tensor_scalar_max(sacc[:, :], sacc[:, :], 1e-30)
            nc.vector.reciprocal(rs[:, :], sacc[:, :])
            # x1 = relu(0.5 * (acc_h0 * rs0 + acc_h1 * rs1))
            tmp = small.tile([128, HID], F32, tag="tmp", name="tmp")
            nc.scalar.mul(tmp[:, :], acc[:, 1, :], rs[:, 1:2])
            xs = small.tile([128, HID], F32, tag="xs", name="xs")
            nc.vector.scalar_tensor_tensor(
                xs[:, :], acc[:, 0, :], rs[:, 0:1], tmp[:, :],
                op0=mybir.AluOpType.mult, op1=mybir.AluOpType.add)
            nc.scalar.activation(x1_all[:, b, :], xs[:, :],
                                 mybir.ActivationFunctionType.Relu, scale=0.5)

        # ---------------- layer-1 node matmuls -------------------------------
        for b in range(B):
            ps_t1 = psum.tile([64, 128], F32, tag="tp")
            nc.tensor.transpose(ps_t1[:, :], x1_all[:, b, :], ident[:, :])
            xt1 = stage.tile([64, 128], F32, tag="xt1")
            nc.vector.tensor_copy(xt1[:, :], ps_t1[:, :])
            ps_m1 = psum.tile([128, REC1], F32, tag="mm")
            nc.tensor.matmul(ps_m1[:, :], xt1[:, :], wcat1[:, :],
                             start=True, stop=True)
            t1b = stage.tile([128, REC1], F32, tag="t1b")
            nc.vector.tensor_copy(t1b[:, :], ps_m1[:, :])
            nc.vector.tensor_copy(a1_sb[:, b, :], ps_m1[:, 4:6])
            rows = min(128, S - b * 128)
            nc.sync.dma_start(t1slice[b * 128:b * 128 + rows, :], t1b[0:rows, :])

        nc.gpsimd.collective_compute(
            "AllGather", mybir.AluOpType.bypass, replica_groups=rg,
            ins=[t1slice[:, :]], outs=[t1tab[:, :]])

        # ---------------- phase 3: layer-1 edge phase ------------------------
        acc_of1 = {}
        for it in sched.items:
            b = it["b"]
            K = it["klo_n"] + it["khi_n"]
            if K == 0:
                nc.vector.memset(out_sb[:, b:b + 1], 0.0)
                continue
            g1 = g1pool.tile([128, K, REC1], F32, tag="g1", name="g1")
            gathers(g1, t1tab, REC1, it)
            w1, s1_i = softmax_w(g1, it, K, 2, a1_sb, REC1)

            pm = small.tile([128, HEADS, K], F32, tag="pm", name="pm")
            nc.vector.tensor_tensor(
                pm[:, :, :], w1[:, :, :],
                g1[:, :, 0:2].rearrange("p k h -> p h k"),
                op=mybir.AluOpType.mult)
            if it["first"]:
                sacc1 = accp.tile([128, HEADS], F32, tag="sa1", name="sacc1")
                acc1 = accp.tile([128, HEADS], F32, tag="aa1", name="acc1")
                acc_of1[b] = (sacc1, acc1)
                nc.vector.tensor_copy(sacc1[:, :], s1_i[:, :])
                nc.vector.reduce_sum(acc1[:, :], pm[:, :, :],
                                     axis=mybir.AxisListType.X)
            else:
                sacc1, acc1 = acc_of1[b]
                nc.vector.tensor_add(sacc1[:, :], sacc1[:, :], s1_i[:, :])
                a1i = small.tile([128, HEADS], F32, tag="a1i", name="a1i")
                nc.vector.reduce_sum(a1i[:, :], pm[:, :, :],
                                     axis=mybir.AxisListType.X)
                nc.vector.tensor_add(acc1[:, :], acc1[:, :], a1i[:, :])
            if not it["last"]:
                continue
            sacc1, acc1 = acc_of1.pop(b)
            rs1 = small.tile([128, HEADS], F32, tag="rs", name="rs1")
            nc.vector.tensor_scalar_max(sacc1[:, :], sacc1[:, :], 1e-30)
            nc.vector.reciprocal(rs1[:, :], sacc1[:, :])
            tmp1 = small.tile([128, 1], F32, tag="tmp1", name="tmp1")
            nc.scalar.mul(tmp1[:, :], acc1[:, 1:2], rs1[:, 1:2])
            oo = small.tile([128, 1], F32, tag="oo", name="oo")
            nc.vector.scalar_tensor_tensor(
                oo[:, :], acc1[:, 0:1], rs1[:, 0:1], tmp1[:, :],
                op0=mybir.AluOpType.mult, op1=mybir.AluOpType.add)
            nc.scalar.activation(out_sb[:, b:b + 1], oo[:, :],
                                 mybir.ActivationFunctionType.Copy, scale=0.5)

        nc.sync.dma_start(out_d[:, :], out_sb[:, :])

    nc.compile()
    return nc


def assemble_output(sched, core_outs, unperm):
    """core_outs: list of [128, B] arrays -> full [N, 1] output."""
    full = np.concatenate(
        [co.T.reshape(-1)[:sched.S] for co in core_outs])   # permuted order
    res = np.empty((sched.N, 1), np.float32)
    res[unperm] = full[:, None]
    return res


# ----------------------------------------------------------------------------
# Harness entry point
# ----------------------------------------------------------------------------

_CACHE = {}


def kernel(x, edge_index, W0, attn0, W1, attn1):
    """Full-input / full-output GAT forward on 8 TRN2 cores."""
    from concourse.bass_interp import get_hw_module
    from concourse.bass_utils import run_bass_kernel_spmd

    NC = 8
    x = np.asarray(x, np.float32)
    edge_index = np.asarray(edge_index)
    sched, per_core, unperm = build_host_data(
        x, edge_index, np.asarray(W0, np.float32), np.asarray(attn0, np.float32),
        np.asarray(W1, np.float32), np.asarray(attn1, np.float32), NC=NC)

    key = (sched.N, sched.W, sched.MW, tuple(sched.K_lo), tuple(sched.K_hi))
    nc = _CACHE.get(key)
    if nc is None:
        nc = build_kernel(sched)
        nc.m = get_hw_module(nc.m)
        _CACHE[key] = nc

    res = run_bass_kernel_spmd(nc, per_core, core_ids=list(range(NC)),
                               trace=False)
    outs = [res.results[r]["out"] for r in range(NC)]
    return assemble_output(sched, outs, unperm)



# revision 2
# speedup vs baseline: 1.0809x; 1.0809x over previous
"""
Self-contained Bass/Trainium2 kernel for the 2-layer 2-head GAT
(nn_GATNet): kernel(**inputs) takes the FULL unsharded inputs
(x [50000,128] f32, edge_index [2,800000] int64, W0, attn0, W1, attn1)
and returns the FULL [50000, 1] f32 output, computed on 8 TRN2
NeuronCores via bass_utils.run_bass_kernel_spmd.
"""
import sys
if "/opt/trn_rl_repo" not in sys.path:
    sys.path.insert(0, "/opt/trn_rl_repo")

"""
GAT (2-layer, 2-head) Bass/Tile kernel for TRN2, 8-core SPMD.

Strategy
--------
Destination-node sharding: core r owns destination nodes [r*S, (r+1)*S).
All edges pointing into that slice are processed by that core, so the
segment softmax and the weighted scatter are purely core-local; the only
collectives are AllGathers of the (node-major) gather tables.

Per layer, node records live in a DRAM table in *permuted position* order
(within each core's slice, nodes are sorted by (deg_lo, deg_hi) so that
padded-CSR batches are uniform):
  L0: Htab [N, 256] bf16 (row = [h0(128), a_c0(2), junk], 512B)
  L1: T1tab [N, 64] f32  (row = [h1(2), a_c1(2), a_r1(2), junk], 256B)

Edge phase (per batch of 128 destination nodes, K slots per node):
  G = dma_gather(table, idx)                  # [128, K, rec] (4 SWDGE queues)
  t = a_c(G) + amask                          # amask = a_r(dest) + mask,
                                              # pre-expanded via Sel matmul
  w = exp(max(t, 0.2*t))                      # softmax numerator
  s = sum_k w ; acc = sum_k w * feat(G)
  out = (acc_h0/max(s_h0,eps) + acc_h1/max(s_h1,eps)) / 2   # head mean

The softmax max-subtraction is dropped (mathematically identical; |t|<~10
for this data so no overflow).  Padding slots gather row 0 and get
t = -1e30 from the additive mask => w == 0 exactly.

int16 gather indices only reach 32767, so each batch gathers twice:
a lo half (table rows 0..N/2-1) and a hi half (rows N/2..N-1).
"""

import math
from contextlib import ExitStack
from dataclasses import dataclass, field

import numpy as np

import concourse.bass as bass
import concourse.bacc as bacc
import concourse.mybir as mybir
import concourse.tile as tile
from concourse import masks

F32 = mybir.dt.float32
BF16 = mybir.dt.bfloat16
I16 = mybir.dt.int16

IN_CH = 128
HID = 64
HEADS = 2
OUT_CH = 1
REC0 = 256                  # 512B bf16 gather record: [h0(128), a_c(2), junk]
REC0F = 132                 # f32 matmul cols: [h0(128), a_c(2), a_r(2)]
REC1 = 64                   # 256B f32 gather record for layer 1
NEG = -1.0e30


# ----------------------------------------------------------------------------
# Host-side preprocessing
# ----------------------------------------------------------------------------

@dataclass
class Schedule:
    N: int
    NC: int
    S: int                      # nodes per core
    B: int                      # batches per core
    S_pad: int                  # B*128
    half: int                   # N // 2
    K_lo: list = field(default_factory=list)   # per-batch lo slot count
    K_hi: list = field(default_factory=list)
    off_lo: list = field(default_factory=list)  # free-dim offset into idx tile
    off_hi: list = field(default_factory=list)
    moff: list = field(default_factory=list)    # free-dim offset into mask tile
    W: int = 0                  # idx tile free dim
    MW: int = 0                 # mask tile free dim
    items: list = field(default_factory=list)   # K-capped pipeline items


def build_host_data(x, edge_index, W0, attn0, W1, attn1, NC=8):
    """Returns (schedule, per_core_inputs, unperm) where unperm[g] is the
    original node id at permuted global position g."""
    x = np.asarray(x, np.float32)
    edge_index = np.asarray(edge_index)
    W0 = np.asarray(W0, np.float32)
    attn0 = np.asarray(attn0, np.float32)
    W1 = np.asarray(W1, np.float32)
    attn1 = np.asarray(attn1, np.float32)

    N = x.shape[0]
    assert N % (2 * NC) == 0, (N, NC)
    S = N // NC
    B = (S + 127) // 128
    S_pad = B * 128
    half = N // 2

    row = edge_index[0].astype(np.int64)
    col = edge_index[1].astype(np.int64)

    sched = Schedule(N=N, NC=NC, S=S, B=B, S_pad=S_pad, half=half)
    assert B <= 64

    # ---- per-core permutation (sort by (deg_lo, deg_hi)) ----
    perms = []
    pos = np.empty(N, np.int64)
    core_edges = []
    for r in range(NC):
        lo_n, hi_n = r * S, (r + 1) * S
        m = (row >= lo_n) & (row < hi_n)
        er, ec = row[m] - lo_n, col[m]
        elo = ec < half
        deg_lo = np.bincount(er[elo], minlength=S)
        deg_hi = np.bincount(er[~elo], minlength=S)
        key = deg_lo.astype(np.int64) * 100000 + deg_hi
        perm = np.argsort(key, kind="stable")
        rank_of = np.empty(S, np.int64)
        rank_of[perm] = np.arange(S)
        perms.append(perm)
        pos[lo_n:hi_n] = r * S + rank_of
        core_edges.append((rank_of[er], ec, elo))

    unperm = np.empty(N, np.int64)
    unperm[pos] = np.arange(N)

    # ---- per-core padded-CSR slot layout (shared K per batch across cores) --
    all_lists = []
    deg_lo_mat = np.zeros((NC, S_pad), np.int64)
    deg_hi_mat = np.zeros((NC, S_pad), np.int64)
    for r in range(NC):
        er, ec, elo = core_edges[r]
        src_pos = pos[ec]
        lists_lo = [[] for _ in range(S_pad)]
        lists_hi = [[] for _ in range(S_pad)]
        for q, p, lo in zip(er, src_pos, elo):
            (lists_lo if lo else lists_hi)[q].append(p)
        for q in range(S):
            deg_lo_mat[r, q] = len(lists_lo[q])
            deg_hi_mat[r, q] = len(lists_hi[q])
        all_lists.append((lists_lo, lists_hi))

    off = 0
    moff = 0
    for b in range(B):
        sl = slice(b * 128, (b + 1) * 128)
        klo = int(deg_lo_mat[:, sl].max(axis=1).max())
        khi = int(deg_hi_mat[:, sl].max(axis=1).max())
        sched.K_lo.append(klo)
        sched.K_hi.append(khi)
        sched.off_lo.append(off)
        off += 8 * klo
        sched.off_hi.append(off)
        off += 8 * khi
        sched.moff.append(moff)
        moff += klo + khi
    sched.W = max(off, 16)
    sched.MW = max(moff, 1)

    # split heavy batches into pipeline items with K <= KCAP
    KCAP = 26
    items = []
    for b in range(B):
        klo, khi = sched.K_lo[b], sched.K_hi[b]
        cuts = []
        done_lo = done_hi = 0
        while done_lo < klo or done_hi < khi:
            take_lo = min(KCAP, klo - done_lo)
            take_hi = min(KCAP - take_lo, khi - done_hi)
            cuts.append((done_lo, take_lo, done_hi, take_hi))
            done_lo += take_lo
            done_hi += take_hi
        for i, (ls, ln, hs, hn) in enumerate(cuts):
            items.append(dict(b=b, klo_s=ls, klo_n=ln, khi_s=hs, khi_n=hn,
                              first=(i == 0), last=(i == len(cuts) - 1)))
    sched.items = items

    # per-item mask offsets (item-ordered contiguous [lo_n | hi_n] blocks)
    imoff = 0
    for it in sched.items:
        it["moff"] = imoff
        imoff += it["klo_n"] + it["khi_n"]
    assert imoff == sched.MW or sched.MW == 1

    # ---- build per-core idx / mneg tiles + permuted x slices ----
    per_core = []
    for r in range(NC):
        lists_lo, lists_hi = all_lists[r]
        idx = np.zeros((16, sched.W), np.int16)
        mneg = np.full((128, sched.MW), NEG, np.float32)
        for b in range(B):
            for (kk, offs, lists, is_hi) in (
                (sched.K_lo[b], sched.off_lo[b], lists_lo, False),
                (sched.K_hi[b], sched.off_hi[b], lists_hi, True),
            ):
                if kk == 0:
                    continue
                blk = np.zeros(kk * 128, np.int64)
                for p in range(128):
                    lst = lists[b * 128 + p]
                    for k, src in enumerate(lst):
                        blk[k * 128 + p] = (src - half) if is_hi else src
                assert blk.max() <= 32767
                idx[:, offs:offs + 8 * kk] = blk.reshape(8 * kk, 16).T
        # item-ordered additive mask
        for it in sched.items:
            b = it["b"]
            col = it["moff"]
            for (ks, kn, lists) in ((it["klo_s"], it["klo_n"], lists_lo),
                                    (it["khi_s"], it["khi_n"], lists_hi)):
                for p in range(128):
                    d = len(lists[b * 128 + p])
                    nreal = max(0, min(d - ks, kn))
                    mneg[p, col:col + nreal] = 0.0
                col += kn
        idx_tile = np.broadcast_to(
            idx[None, :, :], (8, 16, sched.W)).reshape(128, sched.W).copy()

        xp = np.zeros((S_pad, IN_CH), np.float32)
        xp[:S] = x[r * S + perms[r]]
        per_core.append({"xp": xp, "idx": np.ascontiguousarray(idx_tile),
                         "mneg": mneg})

    # ---- Sel matrix: Sel[b, col] = 1 iff mask column col belongs to batch b
    sel = np.zeros((128, sched.MW), np.float32)
    for it in sched.items:
        k = it["klo_n"] + it["khi_n"]
        sel[it["b"], it["moff"]:it["moff"] + k] = 1.0

    # ---- weights / constants (shared across cores) ----
    # wcat0 cols: [0:128]=W0(h0), [128:130]=a_c0 fold, [130:132]=a_r0 fold
    wcat0 = np.zeros((IN_CH, REC0F), np.float32)
    wcat0[:, :128] = W0
    for h in range(HEADS):
        blk = W0[:, h * HID:(h + 1) * HID].astype(np.float64)
        wcat0[:, 128 + h] = (blk @ attn0[h, HID:].astype(np.float64)).astype(np.float32)
        wcat0[:, 130 + h] = (blk @ attn0[h, :HID].astype(np.float64)).astype(np.float32)

    # wcat1 cols: [0:2]=h1, [2:4]=a_c1 fold, [4:6]=a_r1 fold, rest 0
    wcat1 = np.zeros((HID, REC1), np.float32)
    wcat1[:, 0:2] = W1
    for h in range(HEADS):
        wcat1[:, 2 + h] = W1[:, h] * attn1[h, 1]
        wcat1[:, 4 + h] = W1[:, h] * attn1[h, 0]

    for d in per_core:
        d.update({"wcat0": wcat0, "wcat1": wcat1, "sel": sel})
    return sched, per_core, unperm


# ----------------------------------------------------------------------------
# Device kernel builder
# ----------------------------------------------------------------------------

def build_kernel(sched: Schedule, gbufs=5):
    N, NC, B, half, S, S_pad, W = (sched.N, sched.NC, sched.B, sched.half,
                                   sched.S, sched.S_pad, sched.W)
    MW = sched.MW
    nc = bacc.Bacc("TRN2", target_bir_lowering=False, debug=False,
                   num_devices=NC, num_swdge_queues=4)

    xp_d = nc.dram_tensor("xp", [S_pad, IN_CH], F32, kind="ExternalInput")
    idx_d = nc.dram_tensor("idx", [128, W], I16, kind="ExternalInput")
    mneg_d = nc.dram_tensor("mneg", [128, MW], F32, kind="ExternalInput")
    wcat0_d = nc.dram_tensor("wcat0", [IN_CH, REC0F], F32, kind="ExternalInput")
    wcat1_d = nc.dram_tensor("wcat1", [HID, REC1], F32, kind="ExternalInput")
    sel_d = nc.dram_tensor("sel", [128, MW], F32, kind="ExternalInput")
    out_d = nc.dram_tensor("out", [128, B], F32, kind="ExternalOutput")

    rg = [list(range(NC))]
    qn = [0]

    def next_q():
        q = qn[0] % 4
        qn[0] += 1
        return q

    with tile.TileContext(nc) as tc, ExitStack() as ctx:
        aspace = "Shared" if NC > 4 else "Local"
        dram = ctx.enter_context(tc.tile_pool(name="dram", bufs=1, space="DRAM"))
        hslice = dram.tile([S, REC0], BF16)
        htab = dram.tile([N, REC0], BF16, addr_space=aspace)
        t1slice = dram.tile([S, REC1], F32)
        t1tab = dram.tile([N, REC1], F32, addr_space=aspace)

        const = ctx.enter_context(tc.tile_pool(name="const", bufs=1))
        wcat0 = const.tile([IN_CH, REC0F], F32)
        wcat1 = const.tile([HID, REC1], F32)
        ident = const.tile([128, 128], F32)
        idx_sb = const.tile([128, W], I16)
        mneg_sb = const.tile([128, MW], F32)
        sel_sb = const.tile([128, MW], F32)
        a0pad = const.tile([128, 128], F32)   # cols 0:B = a_r head0, 64:64+B = head1
        a1pad = const.tile([128, 128], F32)
        amask0 = const.tile([128, HEADS, MW], BF16)
        amask1 = const.tile([128, HEADS, MW], F32)
        out_sb = const.tile([128, B], F32)
        x1_all = const.tile([128, B, HID], F32)

        nc.sync.dma_start(wcat0[:, :], wcat0_d[:, :])
        nc.sync.dma_start(wcat1[:, :], wcat1_d[:, :])
        nc.sync.dma_start(idx_sb[:, :], idx_d[:, :])
        nc.sync.dma_start(mneg_sb[:, :], mneg_d[:, :])
        nc.sync.dma_start(sel_sb[:, :], sel_d[:, :])
        masks.make_identity(nc, ident[:, :])

        xin = ctx.enter_context(tc.tile_pool(name="xin", bufs=3))
        stage = ctx.enter_context(tc.tile_pool(name="stage", bufs=3))
        psum = ctx.enter_context(tc.tile_pool(name="psum", bufs=2, space="PSUM"))

        # ---------------- phase 1: H table (layer-0 node matmul) -------------
        for t in range(B):
            rows = min(128, S - t * 128)
            x_t = xin.tile([128, IN_CH], F32, tag="x")
            nc.sync.dma_start(x_t[:, :], xp_d[t * 128:(t + 1) * 128, :])
            ps_tr = psum.tile([128, 128], F32, tag="tp")
            nc.tensor.transpose(ps_tr[:, :], x_t[:, :], ident[:, :])
            xt = stage.tile([128, 128], F32, tag="xt")
            nc.scalar.activation(xt[:, :], ps_tr[:, :],
                                 mybir.ActivationFunctionType.Copy)
            ps_mm = psum.tile([128, REC0F], F32, tag="mm")
            nc.tensor.matmul(ps_mm[:, :], xt[:, :], wcat0[:, :],
                             start=True, stop=True)
            ht = stage.tile([128, REC0], BF16, tag="ht")
            nc.vector.tensor_copy(ht[:, 0:130], ps_mm[:, 0:130])
            nc.scalar.activation(a0pad[:, t:t + 1], ps_mm[:, 130:131],
                                 mybir.ActivationFunctionType.Copy)
            nc.scalar.activation(a0pad[:, 64 + t:65 + t], ps_mm[:, 131:132],
                                 mybir.ActivationFunctionType.Copy)
            nc.sync.dma_start(hslice[t * 128:t * 128 + rows, :],
                              ht[0:rows, :])

        nc.gpsimd.collective_compute(
            "AllGather", mybir.AluOpType.bypass, replica_groups=rg,
            ins=[hslice[:, :]], outs=[htab[:, :]])

        # ---- amask0 = expand(a_r0) + mneg, via Sel matmul ----
        def build_amask(apad, amask):
            for h in range(HEADS):
                ps_a = psum.tile([64, 128], F32, tag="tp2")
                nc.tensor.transpose(ps_a[0:64, :],
                                    apad[:, 64 * h:64 * h + 64], ident[:, :])
                aT = stage.tile([64, 128], F32, tag="aT")
                nc.vector.tensor_copy(aT[:, :], ps_a[:, :])
                c0 = 0
                while c0 < MW:
                    wd = min(512, MW - c0)
                    psm = psum.tile([128, 512], F32, tag="mm2")
                    nc.tensor.matmul(psm[:, 0:wd], aT[0:B, :],
                                     sel_sb[0:B, c0:c0 + wd],
                                     start=True, stop=True)
                    nc.vector.tensor_tensor(
                        amask[:, h, c0:c0 + wd], psm[:, 0:wd],
                        mneg_sb[:, c0:c0 + wd], op=mybir.AluOpType.add)
                    c0 += wd

        build_amask(a0pad, amask0)

        # ---------------- edge-phase machinery -------------------------------
        gpool = ctx.enter_context(tc.tile_pool(name="gpool", bufs=gbufs))
        ppool = ctx.enter_context(tc.tile_pool(name="ppool", bufs=3))
        g1pool = ctx.enter_context(tc.tile_pool(name="g1pool", bufs=5))
        small = ctx.enter_context(tc.tile_pool(name="small", bufs=4))
        accp = ctx.enter_context(tc.tile_pool(name="accp", bufs=3))

        def gathers(g, table, rec, it):
            b = it["b"]
            klo, khi = it["klo_n"], it["khi_n"]
            if klo:
                o = sched.off_lo[b] + 8 * it["klo_s"]
                nc.gpsimd.dma_gather(
                    g[:, 0:klo, :], table[0:half, :],
                    idx_sb[:, o:o + 8 * klo],
                    num_idxs=128 * klo, num_idxs_reg=128 * klo, elem_size=rec,
                    single_packet=False, queue_num=next_q())
            if khi:
                o = sched.off_hi[b] + 8 * it["khi_s"]
                nc.gpsimd.dma_gather(
                    g[:, klo:klo + khi, :], table[half:N, :],
                    idx_sb[:, o:o + 8 * khi],
                    num_idxs=128 * khi, num_idxs_reg=128 * khi, elem_size=rec,
                    single_packet=False, queue_num=next_q())

        def softmax_w(g, it, K, ac_col, amask, rec, w_dtype):
            """t = a_c + amask ; w = exp(max(t, .2t)); s_i per head."""
            tt = small.tile([128, HEADS, K], F32, tag="tt", name="tt")
            nc.vector.tensor_tensor(
                tt[:, :, :],
                bass.AP(g.tensor, g.offset + ac_col,
                        [g.ap[0], [1, HEADS], [rec, K]]),
                amask[:, :, it["moff"]:it["moff"] + K],
                op=mybir.AluOpType.add)
            lr = small.tile([128, HEADS, K], F32, tag="lr", name="lr")
            nc.vector.scalar_tensor_tensor(
                lr[:, :, :], tt[:, :, :], 0.2, tt[:, :, :],
                op0=mybir.AluOpType.mult, op1=mybir.AluOpType.max)
            w = small.tile([128, HEADS, K], w_dtype, tag="w", name="w")
            s_i = small.tile([128, HEADS], F32, tag="s", name="s_i")
            for h in range(HEADS):
                nc.scalar.activation(w[:, h, :], lr[:, h, :],
                                     mybir.ActivationFunctionType.Exp,
                                     accum_out=s_i[:, h:h + 1])
            return w, s_i

        # ---------------- phase 2: layer-0 edge phase ------------------------
        acc_of = {}
        for it in sched.items:
            b = it["b"]
            K = it["klo_n"] + it["khi_n"]
            if K == 0:
                nc.vector.memset(x1_all[:, b, :], 0.0)
                continue
            g = gpool.tile([128, K, REC0], BF16, tag="g", name="g")
            gathers(g, htab, REC0, it)
            w, s_i = softmax_w(g, it, K, 128, amask0, REC0, BF16)

            pt2 = ppool.tile([128, K * 128], BF16, tag="p", name="pt2")
            nc.vector.tensor_tensor(
                pt2[:, :].rearrange("p (k h c) -> p k h c", k=K, h=HEADS),
                bass.AP(g.tensor, g.offset,
                        [g.ap[0], [REC0, K], [HID, HEADS], [1, HID]]),
                w[:, :, :].rearrange("p h k -> p k h").unsqueeze(3)
                          .broadcast_to([128, K, HEADS, HID]),
                op=mybir.AluOpType.mult)
            if it["first"]:
                sacc = accp.tile([128, HEADS], F32, tag="sa", name="sacc")
                acc = accp.tile([128, HEADS, HID], F32, tag="aa", name="acc")
                acc_of[b] = (sacc, acc)
                nc.vector.tensor_copy(sacc[:, :], s_i[:, :])
                nc.vector.reduce_sum(
                    acc[:, :, :],
                    pt2[:, :].rearrange("p (k h c) -> p h c k", k=K, h=HEADS),
                    axis=mybir.AxisListType.X)
            else:
                sacc, acc = acc_of[b]
                nc.vector.tensor_add(sacc[:, :], sacc[:, :], s_i[:, :])
                acc_i = small.tile([128, HEADS, HID], F32, tag="ai", name="acc_i")
                nc.vector.reduce_sum(
                    acc_i[:, :, :],
                    pt2[:, :].rearrange("p (k h c) -> p h c k", k=K, h=HEADS),
                    axis=mybir.AxisListType.X)
                nc.vector.tensor_add(acc[:, :, :], acc[:, :, :], acc_i[:, :, :])
            if not it["last"]:
                continue
            sacc, acc = acc_of.pop(b)
            rs = small.tile([128, HEADS], F32, tag="rs", name="rs")
            nc.vector.tensor_scalar_max(sacc[:, :], sacc[:, :], 1e-30)
            nc.vector.reciprocal(rs[:, :], sacc[:, :])
            # x1 = relu(0.5 * (acc_h0 * rs0 + acc_h1 * rs1))
            tmp = small.tile([128, HID], F32, tag="tmp", name="tmp")
            nc.scalar.mul(tmp[:, :], acc[:, 1, :], rs[:, 1:2])
            xs = small.tile([128, HID], F32, tag="xs", name="xs")
            nc.vector.scalar_tensor_tensor(
                xs[:, :], acc[:, 0, :], rs[:, 0:1], tmp[:, :],
                op0=mybir.AluOpType.mult, op1=mybir.AluOpType.add)
            nc.scalar.activation(x1_all[:, b, :], xs[:, :],
                                 mybir.ActivationFunctionType.Relu, scale=0.5)

        # ---------------- layer-1 node matmuls -------------------------------
        for b in range(B):
            ps_t1 = psum.tile([64, 128], F32, tag="tp")
            nc.tensor.transpose(ps_t1[:, :], x1_all[:, b, :], ident[:, :])
            xt1 = stage.tile([64, 128], F32, tag="xt1")
            nc.scalar.activation(xt1[:, :], ps_t1[:, :],
                                 mybir.ActivationFunctionType.Copy)
            ps_m1 = psum.tile([128, REC1], F32, tag="mm")
            nc.tensor.matmul(ps_m1[:, :], xt1[:, :], wcat1[:, :],
                             start=True, stop=True)
            t1b = stage.tile([128, REC1], F32, tag="t1b")
            nc.vector.tensor_copy(t1b[:, 0:6], ps_m1[:, 0:6])
            nc.scalar.activation(a1pad[:, b:b + 1], ps_m1[:, 4:5],
                                 mybir.ActivationFunctionType.Copy)
            nc.scalar.activation(a1pad[:, 64 + b:65 + b], ps_m1[:, 5:6],
                                 mybir.ActivationFunctionType.Copy)
            rows = min(128, S - b * 128)
            nc.sync.dma_start(t1slice[b * 128:b * 128 + rows, :], t1b[0:rows, :])

        nc.gpsimd.collective_compute(
            "AllGather", mybir.AluOpType.bypass, replica_groups=rg,
            ins=[t1slice[:, :]], outs=[t1tab[:, :]])

        build_amask(a1pad, amask1)

        # ---------------- phase 3: layer-1 edge phase ------------------------
        acc_of1 = {}
        for it in sched.items:
            b = it["b"]
            K = it["klo_n"] + it["khi_n"]
            if K == 0:
                nc.vector.memset(out_sb[:, b:b + 1], 0.0)
                continue
            g1 = g1pool.tile([128, K, REC1], F32, tag="g1", name="g1")
            gathers(g1, t1tab, REC1, it)
            w1, s1_i = softmax_w(g1, it, K, 2, amask1, REC1, F32)

            pm = small.tile([128, HEADS, K], F32, tag="pm", name="pm")
            nc.vector.tensor_tensor(
                pm[:, :, :], w1[:, :, :],
                g1[:, :, 0:2].rearrange("p k h -> p h k"),
                op=mybir.AluOpType.mult)
            if it["first"]:
                sacc1 = accp.tile([128, HEADS], F32, tag="sa1", name="sacc1")
                acc1 = accp.tile([128, HEADS], F32, tag="aa1", name="acc1")
                acc_of1[b] = (sacc1, acc1)
                nc.vector.tensor_copy(sacc1[:, :], s1_i[:, :])
                nc.vector.reduce_sum(acc1[:, :], pm[:, :, :],
                                     axis=mybir.AxisListType.X)
            else:
                sacc1, acc1 = acc_of1[b]
                nc.vector.tensor_add(sacc1[:, :], sacc1[:, :], s1_i[:, :])
                a1i = small.tile([128, HEADS], F32, tag="a1i", name="a1i")
                nc.vector.reduce_sum(a1i[:, :], pm[:, :, :],
                                     axis=mybir.AxisListType.X)
                nc.vector.tensor_add(acc1[:, :], acc1[:, :], a1i[:, :])
            if not it["last"]:
                continue
            sacc1, acc1 = acc_of1.pop(b)
            rs1 = small.tile([128, HEADS], F32, tag="rs", name="rs1")
            nc.vector.tensor_scalar_max(sacc1[:, :], sacc1[:, :], 1e-30)
            nc.vector.reciprocal(rs1[:, :], sacc1[:, :])
            tmp1 = small.tile([128, 1], F32, tag="tmp1", name="tmp1")
            nc.scalar.mul(tmp1[:, :], acc1[:, 1:2], rs1[:, 1:2])
            oo = small.tile([128, 1], F32, tag="oo", name="oo")
            nc.vector.scalar_tensor_tensor(
                oo[:, :], acc1[:, 0:1], rs1[:, 0:1], tmp1[:, :],
                op0=mybir.AluOpType.mult, op1=mybir.AluOpType.add)
            nc.scalar.activation(out_sb[:, b:b + 1], oo[:, :],
                                 mybir.ActivationFunctionType.Copy, scale=0.5)

        nc.sync.dma_start(out_d[:, :], out_sb[:, :])

    nc.compile()
    return nc


def assemble_output(sched, core_outs, unperm):
    """core_outs: list of [128, B] arrays -> full [N, 1] output."""
    full = np.concatenate(
        [co.T.reshape(-1)[:sched.S] for co in core_outs])   # permuted order
    res = np.empty((sched.N, 1), np.float32)
    res[unperm] = full[:, None]
    return res


# ----------------------------------------------------------------------------
# Harness entry point
# ----------------------------------------------------------------------------

_CACHE = {}


def kernel(x, edge_index, W0, attn0, W1, attn1):
    """Full-input / full-output GAT forward on 8 TRN2 cores."""
    from concourse.bass_interp import get_hw_module
    from concourse.bass_utils import run_bass_kernel_spmd

    NC = 8
    x = np.asarray(x, np.float32)
    edge_index = np.asarray(edge_index)
    sched, per_core, unperm = build_host_data(
        x, edge_index, np.asarray(W0, np.float32), np.asarray(attn0, np.float32),
        np.asarray(W1, np.float32), np.asarray(attn1, np.float32), NC=NC)

    key = (sched.N, sched.W, sched.MW, tuple(sched.K_lo), tuple(sched.K_hi))
    nc = _CACHE.get(key)
    if nc is None:
        nc = build_kernel(sched)
        nc.m = get_hw_module(nc.m)
        _CACHE[key] = nc

    res = run_bass_kernel_spmd(nc, per_core, core_ids=list(range(NC)),
                               trace=False)
    outs = [res.results[r]["out"] for r in range(NC)]
    return assemble_output(sched, outs, unperm)
